# revision 9
# baseline (speedup 1.0000x reference)
"""Bass/Tile kernel: cosine top-20 adjacency (16384x64 embeddings) on 8 trn2 cores.

Per-core algorithm (rows sharded 2048/core via host-side input rotation, so the
same SPMD graph runs on every core):
  1. Load embeddings with nodes (j, j+8192) paired on the same partition slot
     (emb view "(two q h) d -> q (h two) d"), normalize rows (square ->
     windowed reduce -> sqrt -> reciprocal), fused normalize + fp8e4 cast
     writing (h, two, d) -> [h][d][two] so each (node, node+8192) fp8 pair
     is one uint16.
  2. Round-trip through DRAM; XBAR-transpose the [4096, 128] uint16 view
     (two chunks, ONE DMA ring - concurrent transposes on two rings corrupt
     the XBAR). Result: partitions 0-63 = dims over EVEN nodes, 64-127 =
     ODD nodes, column c -> node (c even: c | c odd: c-1+8192). Four
     partition-shift DMAs assemble fp8p [32, 2*16384]: two K=32 dim-planes
     x (even-node block | odd-node block).
  3. Per 128-row tile (8 even-node + 8 odd-node tiles, lhsT = 128 stride-2
     columns): sim = fp8 DoubleRow matmuls (K=2x32 planes, 0.5 cyc/row,
     2x bf16 PE rate) into PSUM, 4 per 2048-col group, double-buffered.
     Evacuation per GROUP_KIND: "A" = Act casts 2048 f32 -> bf16, DVE
     tensor_max folds halves at its 2x bf16 mode; "S" = Act casts only the
     hi 1024, DVE tensor_max(ps_lo, cast_hi) fuses evacuation+fold (PSUM
     may only feed ONE TT operand). Fold tree L2-L5 on DVE -> 512 win-32
     maxima, max8 per 128-chunk -> 32 candidates, 3x(max8 + match_replace)
     -> top-24 descending.
  4. Self-similarity (~1.0) is always the strict row max, so
     out[:,0] = 0 and out[:,1:20] = sigmoid(top24[:,1:20]).
"""

import os
import sys

import numpy as np

for _p in ("/opt/trn_rl_repo",):
    if _p not in sys.path and os.path.isdir(_p):
        sys.path.insert(0, _p)

import concourse.bass as bass  # noqa: E402
import concourse.mybir as mybir  # noqa: E402
import concourse.tile as tile  # noqa: E402
from concourse import bacc  # noqa: E402
from concourse.bass_utils import run_bass_kernel_spmd  # noqa: E402

N = 16384
D = 64
TOPK = 20
CORES = 8
R = N // CORES  # 2048 rows per core
T = R // 128  # 16 row tiles per core
G = 2048  # column group size
NG = N // G  # 8 column groups
H = N // 2
NEG = -1.0e30

f32 = mybir.dt.float32
bf16 = mybir.dt.bfloat16
fp8 = mybir.dt.float8e4
u16 = mybir.dt.uint16
AF = mybir.ActivationFunctionType
ALU = mybir.AluOpType
PM = mybir.MatmulPerfMode

# "A" = Act casts the whole group; "S" = split: Act casts hi half, DVE
# tensor_max fuses PSUM-lo evacuation with the L1 fold.
GROUP_KIND = ("A", "A", "S", "A", "A", "A", "S", "A")

_CACHE = {}


def _build_nc():
    nc = bacc.Bacc(
        "TRN2", target_bir_lowering=False, debug=False, enable_asserts=False
    )
    emb = nc.dram_tensor("embeddings", [N, D], f32, kind="ExternalInput")
    out = nc.dram_tensor("out", [R, TOPK], f32, kind="ExternalOutput")
    # tile (parity pr, block tb) row s = local node 256*tb + 2*s + pr
    out_v = out[:].rearrange("(n two) k -> two n k", two=2)

    with tile.TileContext(nc) as tc:
        with tc.tile_pool(name="persist", bufs=1) as persist:
            fp8p = persist.tile([32, 2 * N], fp8)
            v = fp8p[:].rearrange("k (two n) -> k two n", two=2)

            # ---- Prologue: normalize, fp8 cast, XBAR transpose, planes ----
            with (
                tc.tile_pool(name="pro_rm", bufs=1) as pro_rm,
                tc.tile_pool(name="pro_t2", bufs=1) as pro_t2,
                tc.tile_pool(name="pro_dram", bufs=1, space="DRAM") as pro_dram,
            ):
                # slot (h, two) on partition q holds node 8192*two+64*q+h
                emb_v = emb[:].rearrange(
                    "(two q h) d -> q h two d", two=2, q=128
                )
                rm = pro_rm.tile([128, 64, 2, D], f32)
                sq = pro_rm.tile([128, 64, 2, D], f32)
                ssq = pro_rm.tile([128, 64, 2], f32)
                slen = pro_rm.tile([128, 64, 2], f32)
                sinv = pro_rm.tile([128, 64, 2], f32)
                rmb8 = pro_rm.tile([128, 64, D, 2], fp8)
                scratch = pro_dram.tile([N // 2, D], u16)
                sc_v = scratch[:].rearrange("(p h) d -> p h d", p=128)
                engs = (nc.sync, nc.scalar)
                NCH = 8
                CW = 128 // NCH
                HW_ = 64 // NCH  # h-slots per chunk
                for c in range(NCH):
                    hs = slice(c * HW_, (c + 1) * HW_)
                    # DMA supports <=3 dims: one transfer per node-half
                    engs[c % 2].dma_start(rm[:, hs, 0, :], emb_v[:, hs, 0, :])
                    engs[c % 2].dma_start(rm[:, hs, 1, :], emb_v[:, hs, 1, :])
                    nc.scalar.activation(sq[:, hs], rm[:, hs], AF.Square)

                def _red(c):
                    hs = slice(c * HW_, (c + 1) * HW_)
                    nc.vector.tensor_reduce(
                        ssq[:, hs], sq[:, hs],
                        axis=mybir.AxisListType.X, op=ALU.add,
                    )

                def _stt(c):
                    hs = slice(c * HW_, (c + 1) * HW_)
                    for tw in range(2):
                        nc.vector.scalar_tensor_tensor(
                            rmb8[:, hs, :, tw],
                            rm[:, hs, tw, :], 1.0,
                            sinv[:, hs, tw].to_broadcast((128, HW_, D)),
                            op0=ALU.mult, op1=ALU.mult,
                        )
                    engs[c % 2].dma_start(
                        sc_v[:, hs, :], rmb8[:, hs].bitcast(u16)
                    )

                for c in range(4):
                    _red(c)
                nc.scalar.activation(slen[:, 0:32], ssq[:, 0:32], AF.Sqrt)
                nc.vector.reciprocal(sinv[:, 0:32], slen[:, 0:32])
                for c in range(4):
                    _stt(c)
                for c in range(4, 8):
                    _red(c)
                nc.scalar.activation(slen[:, 32:64], ssq[:, 32:64], AF.Sqrt)
                nc.vector.reciprocal(sinv[:, 32:64], slen[:, 32:64])
                for c in range(4, 8):
                    _stt(c)

                # XBAR transpose [4096, 128] u16 in two chunks (scalar ring
                # only). nt partition 64*par+d = dim d over par-parity nodes.
                sc_t = scratch[:].rearrange("(r two) d -> r (two d)", two=2)
                nt = pro_t2.tile([128, N // 4], u16)
                ntf = nt[:].bitcast(fp8)  # [128, 8192]
                for c in range(2):
                    ms = slice(c * 2048, (c + 1) * 2048)
                    nc.scalar.dma_start(
                        out=nt[:, ms], in_=sc_t[ms, :], transpose=True
                    )
                # assemble planes: [even block | odd block] per plane
                nc.sync.dma_start(v[:, 0, 0:H], ntf[0:32, :])
                nc.gpsimd.dma_start(v[:, 0, H:N], ntf[64:96, :])
                nc.sync.dma_start(v[:, 1, 0:H], ntf[32:64, :])
                nc.gpsimd.dma_start(v[:, 1, H:N], ntf[96:128, :])

            # ---- Main loop: 16 row tiles ----
            with (
                tc.tile_pool(name="mm_psum", bufs=2, space="PSUM") as mm_psum,
                tc.tile_pool(name="ev_cast", bufs=8) as ev_cast,
                tc.tile_pool(name="ev_ch", bufs=4) as ev_ch,
                tc.tile_pool(name="pyr", bufs=2) as pyr,
                tc.tile_pool(name="fin", bufs=2) as fin,
            ):
                for t in range(T):
                    pr, tb = t // 8, t % 8
                    c0 = pr * H + tb * 256
                    lhsT = v[:, :, c0 : c0 + 256].rearrange(
                        "k two (m s) -> k two m s", s=2
                    )[:, :, :, 0]
                    l1b = pyr.tile([128, NG, G // 2], bf16, tag="l1b")
                    for g in range(NG):
                        ps = mm_psum.tile([128, G], f32, tag="ps")
                        for s in range(G // 512):
                            cl = g * G + s * 512
                            nc.tensor.matmul(
                                ps[:, s * 512 : (s + 1) * 512],
                                lhsT, v[:, :, cl : cl + 512],
                                perf_mode=PM.DoubleRow,
                            )
                        if GROUP_KIND[g] == "S":
                            ch = ev_ch.tile([128, G // 2], bf16, tag="ch")
                            nc.scalar.activation(
                                ch[:], ps[:, G // 2 : G], AF.Copy
                            )
                            nc.vector.tensor_max(
                                l1b[:, g, :], ps[:, 0 : G // 2], ch[:]
                            )
                        else:
                            ca = ev_cast.tile([128, G], bf16, tag="ca")
                            nc.scalar.activation(ca[:], ps[:], AF.Copy)
                            nc.vector.tensor_max(
                                l1b[:, g, :],
                                ca[:, 0 : G // 2], ca[:, G // 2 : G],
                            )

                    # fold tree on DVE -> 512 win-32 maxima
                    f2 = pyr.tile([128, 4, G // 2], bf16, tag="f2")
                    for i in range(4):
                        nc.vector.tensor_max(
                            f2[:, i, :], l1b[:, 2 * i, :], l1b[:, 2 * i + 1, :]
                        )
                    f3 = pyr.tile([128, 2, G // 2], bf16, tag="f3")
                    for i in range(2):
                        nc.vector.tensor_max(
                            f3[:, i, :], f2[:, 2 * i, :], f2[:, 2 * i + 1, :]
                        )
                    f4 = pyr.tile([128, G // 2], bf16, tag="f4")
                    nc.vector.tensor_max(f4[:], f3[:, 0, :], f3[:, 1, :])
                    f5 = pyr.tile([128, G // 4], bf16, tag="f5")
                    nc.vector.tensor_max(
                        f5[:], f4[:, 0 : G // 4], f4[:, G // 4 : G // 2]
                    )

                    # candidates: top-8 of each 128-chunk of the 512 maxima
                    cand = fin.tile([128, 32], bf16, tag="cand")
                    for c in range(4):
                        nc.vector.max(
                            out=cand[:, c * 8 : (c + 1) * 8],
                            in_=f5[:, c * 128 : (c + 1) * 128],
                        )
                    # top-24 via 3x max8 + 2x match_replace
                    top24 = fin.tile([128, 24], bf16, tag="top24")
                    cand2 = fin.tile([128, 32], bf16, tag="cand2")
                    cand3 = fin.tile([128, 32], bf16, tag="cand3")
                    nc.vector.max(out=top24[:, 0:8], in_=cand[:])
                    nc.vector.match_replace(
                        out=cand2[:], in_to_replace=top24[:, 0:8],
                        in_values=cand[:], imm_value=NEG,
                    )
                    nc.vector.max(out=top24[:, 8:16], in_=cand2[:])
                    nc.vector.match_replace(
                        out=cand3[:], in_to_replace=top24[:, 8:16],
                        in_values=cand2[:], imm_value=NEG,
                    )
                    nc.vector.max(out=top24[:, 16:24], in_=cand3[:])

                    # epilogue: out[:,0] = 0; out[:,1:20] = sigmoid(top24[:,1:20])
                    osb = fin.tile([128, TOPK], f32, tag="osb")
                    nc.gpsimd.memset(osb[:, 0:1], 0.0)
                    nc.scalar.activation(
                        osb[:, 1:TOPK], top24[:, 1:TOPK], AF.Sigmoid
                    )
                    nc.sync.dma_start(
                        out_v[pr, tb * 128 : (tb + 1) * 128, :], osb[:]
                    )

    nc.compile()
    return nc


def get_nc():
    if "nc" not in _CACHE:
        _CACHE["nc"] = _build_nc()
    return _CACHE["nc"]


def kernel(embeddings: np.ndarray) -> np.ndarray:
    emb = np.ascontiguousarray(np.asarray(embeddings, dtype=np.float32))
    assert emb.shape == (N, D), emb.shape
    nc = get_nc()
    in_maps = [
        {"embeddings": np.roll(emb, -i * R, axis=0)} for i in range(CORES)
    ]
    res = run_bass_kernel_spmd(nc, in_maps, core_ids=list(range(CORES)))
    _CACHE["last_results"] = res
    return np.concatenate(
        [res.results[i]["out"] for i in range(CORES)], axis=0
    ).astype(np.float32)


# revision 10
# speedup vs baseline: 1.0131x; 1.0131x over previous
"""Bass/Tile kernel: cosine top-20 adjacency (16384x64 embeddings) on 8 trn2 cores.

Per-core algorithm (rows sharded 2048/core via host-side input rotation, so the
same SPMD graph runs on every core):
  1. Load embeddings with nodes (j, j+8192) paired on the same partition slot
     (emb view "(two q h) d -> q (h two) d"), normalize rows (square ->
     windowed reduce -> sqrt -> reciprocal), fused normalize + fp8e4 cast
     writing (h, two, d) -> [h][d][two] so each (node, node+8192) fp8 pair
     is one uint16.
  2. Round-trip through DRAM; XBAR-transpose the [4096, 128] uint16 view
     (two chunks, ONE DMA ring - concurrent transposes on two rings corrupt
     the XBAR). Result: partitions 0-63 = dims over EVEN nodes, 64-127 =
     ODD nodes, column c -> node (c even: c | c odd: c-1+8192). Four
     partition-shift DMAs assemble fp8p [32, 2*16384]: two K=32 dim-planes
     x (even-node block | odd-node block).
  3. Per 128-row tile (8 even-node + 8 odd-node tiles, lhsT = 128 stride-2
     columns): sim = fp8 DoubleRow matmuls (K=2x32 planes, 0.5 cyc/row,
     2x bf16 PE rate) into PSUM, 4 per 2048-col group, double-buffered.
     Evacuation per GROUP_KIND: "A" = Act casts 2048 f32 -> bf16, DVE
     tensor_max folds halves at its 2x bf16 mode; "S" = Act casts only the
     hi 1024, DVE tensor_max(ps_lo, cast_hi) fuses evacuation+fold (PSUM
     may only feed ONE TT operand). Fold tree L2-L5 on DVE -> 512 win-32
     maxima, max8 per 128-chunk -> 32 candidates, 3x(max8 + match_replace)
     -> top-24 descending.
  4. Self-similarity (~1.0) is always the strict row max, so
     out[:,0] = 0 and out[:,1:20] = sigmoid(top24[:,1:20]).
"""

import os
import sys

import numpy as np

for _p in ("/opt/trn_rl_repo",):
    if _p not in sys.path and os.path.isdir(_p):
        sys.path.insert(0, _p)

import concourse.bass as bass  # noqa: E402
import concourse.mybir as mybir  # noqa: E402
import concourse.tile as tile  # noqa: E402
from concourse import bacc  # noqa: E402
from concourse.bass_utils import run_bass_kernel_spmd  # noqa: E402

N = 16384
D = 64
TOPK = 20
CORES = 8
R = N // CORES  # 2048 rows per core
T = R // 128  # 16 row tiles per core
G = 2048  # column group size
NG = N // G  # 8 column groups
H = N // 2
NEG = -1.0e30

f32 = mybir.dt.float32
bf16 = mybir.dt.bfloat16
fp8 = mybir.dt.float8e4
u16 = mybir.dt.uint16
AF = mybir.ActivationFunctionType
ALU = mybir.AluOpType
PM = mybir.MatmulPerfMode

# "A" = Act casts the whole group; "S" = split: Act casts hi half, DVE
# tensor_max fuses PSUM-lo evacuation with the L1 fold.
GROUP_KIND = ("A", "A", "S", "A", "A", "A", "S", "A")

_CACHE = {}


def _build_nc():
    nc = bacc.Bacc(
        "TRN2", target_bir_lowering=False, debug=False, enable_asserts=False
    )
    emb = nc.dram_tensor("embeddings", [N, D], f32, kind="ExternalInput")
    out = nc.dram_tensor("out", [R, TOPK], f32, kind="ExternalOutput")
    # tile (parity pr, block tb) row s = local node 256*tb + 2*s + pr
    out_v = out[:].rearrange("(n two) k -> two n k", two=2)

    with tile.TileContext(nc) as tc:
        with tc.tile_pool(name="persist", bufs=1) as persist:
            fp8p = persist.tile([32, 2 * N], fp8)
            v = fp8p[:].rearrange("k (two n) -> k two n", two=2)

            # ---- Prologue: normalize, fp8 cast, XBAR transpose, planes ----
            with (
                tc.tile_pool(name="pro_rm", bufs=1) as pro_rm,
                tc.tile_pool(name="pro_t2", bufs=1) as pro_t2,
                tc.tile_pool(name="pro_dram", bufs=1, space="DRAM") as pro_dram,
            ):
                # slot (h, two) on partition q holds node 8192*two+64*q+h
                emb_v = emb[:].rearrange(
                    "(two q h) d -> q h two d", two=2, q=128
                )
                rm = pro_rm.tile([128, 64, 2, D], f32)
                sq = pro_rm.tile([128, 64, 2, D], f32)
                ssq = pro_rm.tile([128, 64, 2], f32)
                slen = pro_rm.tile([128, 64, 2], f32)
                sinv = pro_rm.tile([128, 64, 2], f32)
                rmb8 = pro_rm.tile([128, 64, D, 2], fp8)
                scratch = pro_dram.tile([N // 2, D], u16)
                sc_v = scratch[:].rearrange("(p h) d -> p h d", p=128)
                engs = (nc.sync, nc.scalar)
                NCH = 8
                CW = 128 // NCH
                HW_ = 64 // NCH  # h-slots per chunk
                for c in range(NCH):
                    hs = slice(c * HW_, (c + 1) * HW_)
                    # DMA supports <=3 dims: one transfer per node-half
                    engs[c % 2].dma_start(rm[:, hs, 0, :], emb_v[:, hs, 0, :])
                    engs[c % 2].dma_start(rm[:, hs, 1, :], emb_v[:, hs, 1, :])
                    nc.scalar.activation(sq[:, hs], rm[:, hs], AF.Square)

                def _red(c):
                    hs = slice(c * HW_, (c + 1) * HW_)
                    nc.vector.tensor_reduce(
                        ssq[:, hs], sq[:, hs],
                        axis=mybir.AxisListType.X, op=ALU.add,
                    )

                def _stt(c):
                    hs = slice(c * HW_, (c + 1) * HW_)
                    for tw in range(2):
                        nc.vector.scalar_tensor_tensor(
                            rmb8[:, hs, :, tw],
                            rm[:, hs, tw, :], 1.0,
                            sinv[:, hs, tw].to_broadcast((128, HW_, D)),
                            op0=ALU.mult, op1=ALU.mult,
                        )
                    engs[c % 2].dma_start(
                        sc_v[:, hs, :], rmb8[:, hs].bitcast(u16)
                    )

                for c in range(4):
                    _red(c)
                nc.scalar.activation(slen[:, 0:32], ssq[:, 0:32], AF.Sqrt)
                nc.vector.reciprocal(sinv[:, 0:32], slen[:, 0:32])
                for c in range(4):
                    _stt(c)
                for c in range(4, 8):
                    _red(c)
                nc.scalar.activation(slen[:, 32:64], ssq[:, 32:64], AF.Sqrt)
                nc.vector.reciprocal(sinv[:, 32:64], slen[:, 32:64])
                for c in range(4, 8):
                    _stt(c)

                # XBAR transpose [4096, 128] u16 in two chunks (scalar ring
                # only). nt partition 64*par+d = dim d over par-parity nodes.
                sc_t = scratch[:].rearrange("(r two) d -> r (two d)", two=2)
                nt = pro_t2.tile([128, N // 4], u16)
                ntf = nt[:].bitcast(fp8)  # [128, 8192]
                for c in range(2):
                    ms = slice(c * 2048, (c + 1) * 2048)
                    nc.scalar.dma_start(
                        out=nt[:, ms], in_=sc_t[ms, :], transpose=True
                    )
                # assemble planes: [even block | odd block] per plane
                nc.sync.dma_start(v[:, 0, 0:H], ntf[0:32, :])
                nc.gpsimd.dma_start(v[:, 0, H:N], ntf[64:96, :])
                nc.sync.dma_start(v[:, 1, 0:H], ntf[32:64, :])
                nc.gpsimd.dma_start(v[:, 1, H:N], ntf[96:128, :])

            # ---- Main loop: 16 row tiles, DVE software-pipelined one tile ----
            # Only PSUM-touching DVE ops ("S" fused L1) issue during a tile's
            # matmul/evac phase; all SBUF folds + candidate selection for tile
            # t-1 are emitted after tile t's evac ops, so the DVE queue never
            # holds a PSUM bank while grinding a fold tree.
            with (
                tc.tile_pool(name="mm_psum", bufs=2, space="PSUM") as mm_psum,
                tc.tile_pool(name="ev_cast", bufs=14) as ev_cast,
                tc.tile_pool(name="ev_ch", bufs=4) as ev_ch,
                tc.tile_pool(name="pyr", bufs=2) as pyr,
                tc.tile_pool(name="fin", bufs=2) as fin,
            ):

                def emit_mm_evac(t):
                    pr, tb = t // 8, t % 8
                    c0 = pr * H + tb * 256
                    lhsT = v[:, :, c0 : c0 + 256].rearrange(
                        "k two (m s) -> k two m s", s=2
                    )[:, :, :, 0]
                    l1b = pyr.tile([128, NG, G // 2], bf16, tag="l1b")
                    cas = {}
                    for g in range(NG):
                        ps = mm_psum.tile([128, G], f32, tag="ps")
                        for s in range(G // 512):
                            cl = g * G + s * 512
                            nc.tensor.matmul(
                                ps[:, s * 512 : (s + 1) * 512],
                                lhsT, v[:, :, cl : cl + 512],
                                perf_mode=PM.DoubleRow,
                            )
                        if GROUP_KIND[g] == "S":
                            ch = ev_ch.tile([128, G // 2], bf16, tag="ch")
                            nc.scalar.activation(
                                ch[:], ps[:, G // 2 : G], AF.Copy
                            )
                            nc.vector.tensor_max(
                                l1b[:, g, :], ps[:, 0 : G // 2], ch[:]
                            )
                        else:
                            ca = ev_cast.tile([128, G], bf16, tag="ca")
                            nc.scalar.activation(ca[:], ps[:], AF.Copy)
                            cas[g] = ca
                    return (t, l1b, cas)

                def emit_fold_tail(state):
                    t, l1b, cas = state
                    pr, tb = t // 8, t % 8
                    for g, ca in cas.items():
                        nc.vector.tensor_max(
                            l1b[:, g, :],
                            ca[:, 0 : G // 2], ca[:, G // 2 : G],
                        )
                    f2 = pyr.tile([128, 4, G // 2], bf16, tag="f2")
                    for i in range(4):
                        nc.vector.tensor_max(
                            f2[:, i, :], l1b[:, 2 * i, :], l1b[:, 2 * i + 1, :]
                        )
                    f3 = pyr.tile([128, 2, G // 2], bf16, tag="f3")
                    for i in range(2):
                        nc.vector.tensor_max(
                            f3[:, i, :], f2[:, 2 * i, :], f2[:, 2 * i + 1, :]
                        )
                    f4 = pyr.tile([128, G // 2], bf16, tag="f4")
                    nc.vector.tensor_max(f4[:], f3[:, 0, :], f3[:, 1, :])
                    f5 = pyr.tile([128, G // 4], bf16, tag="f5")
                    nc.vector.tensor_max(
                        f5[:], f4[:, 0 : G // 4], f4[:, G // 4 : G // 2]
                    )

                    cand = fin.tile([128, 32], bf16, tag="cand")
                    for c in range(4):
                        nc.vector.max(
                            out=cand[:, c * 8 : (c + 1) * 8],
                            in_=f5[:, c * 128 : (c + 1) * 128],
                        )
                    top24 = fin.tile([128, 24], bf16, tag="top24")
                    cand2 = fin.tile([128, 32], bf16, tag="cand2")
                    cand3 = fin.tile([128, 32], bf16, tag="cand3")
                    nc.vector.max(out=top24[:, 0:8], in_=cand[:])
                    nc.vector.match_replace(
                        out=cand2[:], in_to_replace=top24[:, 0:8],
                        in_values=cand[:], imm_value=NEG,
                    )
                    nc.vector.max(out=top24[:, 8:16], in_=cand2[:])
                    nc.vector.match_replace(
                        out=cand3[:], in_to_replace=top24[:, 8:16],
                        in_values=cand2[:], imm_value=NEG,
                    )
                    nc.vector.max(out=top24[:, 16:24], in_=cand3[:])

                    osb = fin.tile([128, TOPK], f32, tag="osb")
                    nc.gpsimd.memset(osb[:, 0:1], 0.0)
                    nc.scalar.activation(
                        osb[:, 1:TOPK], top24[:, 1:TOPK], AF.Sigmoid
                    )
                    nc.sync.dma_start(
                        out_v[pr, tb * 128 : (tb + 1) * 128, :], osb[:]
                    )

                prev = None
                for t in range(T):
                    cur = emit_mm_evac(t)
                    if prev is not None:
                        emit_fold_tail(prev)
                    prev = cur
                emit_fold_tail(prev)

    nc.compile()
    return nc


def get_nc():
    if "nc" not in _CACHE:
        _CACHE["nc"] = _build_nc()
    return _CACHE["nc"]


def kernel(embeddings: np.ndarray) -> np.ndarray:
    emb = np.ascontiguousarray(np.asarray(embeddings, dtype=np.float32))
    assert emb.shape == (N, D), emb.shape
    nc = get_nc()
    in_maps = [
        {"embeddings": np.roll(emb, -i * R, axis=0)} for i in range(CORES)
    ]
    res = run_bass_kernel_spmd(nc, in_maps, core_ids=list(range(CORES)))
    _CACHE["last_results"] = res
    return np.concatenate(
        [res.results[i]["out"] for i in range(CORES)], axis=0
    ).astype(np.float32)


# revision 11
# speedup vs baseline: 1.1156x; 1.1011x over previous
"""Bass/Tile kernel: cosine top-20 adjacency (16384x64 embeddings) on 8 trn2 cores.

Per-core algorithm (rows sharded 2048/core via host-side input rotation, so the
same SPMD graph runs on every core):
  1. Load embeddings row-major, compute row norms (square -> windowed reduce ->
     sqrt -> reciprocal), fused normalize+bf16-cast.
  2. Round-trip through DRAM and XBAR-transpose the [8192, 128] bf16 view ->
     normT [64, 16384] with columns permuted to [even rows | odd rows]
     (column order is irrelevant: the output is values-only).
     NOTE: both XBAR transposes must stay on ONE DMA ring - running them
     concurrently on different rings silently corrupts the transpose.
  3. Per 128-row tile (t<8: even local rows of band t; t>=8: odd rows):
     sim = lhsT.T @ normT (bf16 matmuls into PSUM, 4 matmuls per 2048-col
     group, PSUM double-buffered). Act casts each group to bf16 and is the
     ONLY reader of PSUM, so PSUM banks recycle at pure Act cadence.
     The DVE work is software-pipelined so it never delays Act or PE:
     iteration i emits [matmuls+casts](i), [L1+fold tree](i-1),
     [cands/top-24/sigmoid/out-DMA](i-2). DVE folds the casts at its 2x
     bf16 mode: L1 pairs halves -> 8x1024, tree -> 512 win-32 maxima,
     max8 per 128-chunk -> 32 candidates, 3x(max8 + match_replace) ->
     top-24 descending.
  4. Self-similarity (~1.0) is always the strict row max, so
     out[:,0] = 0 and out[:,1:20] = sigmoid(top24[:,1:20]).
"""

import os
import sys

import numpy as np

for _p in ("/opt/trn_rl_repo",):
    if _p not in sys.path and os.path.isdir(_p):
        sys.path.insert(0, _p)

import concourse.bass as bass  # noqa: E402
import concourse.mybir as mybir  # noqa: E402
import concourse.tile as tile  # noqa: E402
from concourse import bacc  # noqa: E402
from concourse.bass_utils import run_bass_kernel_spmd  # noqa: E402

N = 16384
D = 64
TOPK = 20
CORES = 8
R = N // CORES  # 2048 rows per core
T = R // 128  # 16 row tiles per core
G = 2048  # column group size
NG = N // G  # 8 column groups
H = N // 2  # even/odd half size in permuted column space
NEG = -1.0e30

f32 = mybir.dt.float32
bf16 = mybir.dt.bfloat16
AF = mybir.ActivationFunctionType
ALU = mybir.AluOpType

_CACHE = {}


def _build_nc():
    nc = bacc.Bacc(
        "TRN2", target_bir_lowering=False, debug=False, enable_asserts=False
    )
    emb = nc.dram_tensor("embeddings", [N, D], f32, kind="ExternalInput")
    out = nc.dram_tensor("out", [R, TOPK], f32, kind="ExternalOutput")
    # tile t<8 covers even local rows 2*(t*128+q), tile t>=8 odd rows.
    out_v = out[:].rearrange("(j two) k -> two j k", two=2)

    with tile.TileContext(nc) as tc:
        with tc.tile_pool(name="persist", bufs=1) as persist:
            normT = persist.tile([D, N], bf16)

            # ---- Prologue: normalize rows, cast bf16, XBAR transpose ----
            with (
                tc.tile_pool(name="pro_rm", bufs=1) as pro_rm,
                tc.tile_pool(name="pro_t2", bufs=1) as pro_t2,
                tc.tile_pool(name="pro_dram", bufs=1, space="DRAM") as pro_dram,
            ):
                # flat [128, 128, 64] staging view: row r = p*128 + a
                emb_v = emb[:].rearrange("(p a) d -> p a d", p=128)
                rm = pro_rm.tile([128, 128, D], f32)
                sq = pro_rm.tile([128, 128, D], f32)
                ssq = pro_rm.tile([128, 128], f32)
                slen = pro_rm.tile([128, 128], f32)
                sinv = pro_rm.tile([128, 128], f32)
                rmb = pro_rm.tile([128, 128, D], bf16)
                scratch = pro_dram.tile([N, D], bf16)
                sc_v = scratch[:].rearrange("(p a) d -> p a d", p=128)
                engs = (nc.sync, nc.scalar)
                NCH = 8
                CW = 128 // NCH
                for c in range(NCH):
                    cs = slice(c * CW, (c + 1) * CW)
                    engs[c % 2].dma_start(rm[:, cs, :], emb_v[:, cs, :])
                    nc.scalar.activation(sq[:, cs, :], rm[:, cs, :], AF.Square)

                def _red(c):
                    cs = slice(c * CW, (c + 1) * CW)
                    nc.vector.tensor_reduce(
                        ssq[:, cs], sq[:, cs, :],
                        axis=mybir.AxisListType.X, op=ALU.add,
                    )

                def _stt(c):
                    cs = slice(c * CW, (c + 1) * CW)
                    nc.vector.scalar_tensor_tensor(
                        rmb[:, cs, :], rm[:, cs, :], 1.0,
                        sinv[:, cs].to_broadcast((128, CW, D)),
                        op0=ALU.mult, op1=ALU.mult,
                    )
                    engs[c % 2].dma_start(sc_v[:, cs, :], rmb[:, cs, :])

                for c in range(4):
                    _red(c)
                nc.scalar.activation(slen[:, 0:64], ssq[:, 0:64], AF.Sqrt)
                nc.vector.reciprocal(sinv[:, 0:64], slen[:, 0:64])
                for c in range(4):
                    _stt(c)
                for c in range(4, 8):
                    _red(c)
                nc.scalar.activation(slen[:, 64:128], ssq[:, 64:128], AF.Sqrt)
                nc.vector.reciprocal(sinv[:, 64:128], slen[:, 64:128])
                for c in range(4, 8):
                    _stt(c)

                # XBAR transpose of the [8192, 128] bf16 view, in two row
                # chunks, BOTH on the scalar ring: nt2 partition c<64 holds
                # column c over even rows, 64+d over odd rows.
                sc_t = scratch[:].rearrange("(m two) d -> m (two d)", two=2)
                nt2 = pro_t2.tile([128, H], bf16)
                for c in range(2):
                    ms = slice(c * 4096, (c + 1) * 4096)
                    nc.scalar.dma_start(
                        out=nt2[:, ms], in_=sc_t[ms, :], transpose=True
                    )
                    nc.vector.tensor_copy(
                        normT[:, c * 4096 : (c + 1) * 4096], nt2[0:D, ms]
                    )
                    nc.sync.dma_start(
                        normT[:, H + c * 4096 : H + (c + 1) * 4096],
                        nt2[D:128, ms],
                    )

            # ---- Main loop: 16 row tiles, DVE pipelined 1-2 tiles back ----
            with (
                tc.tile_pool(name="mm_psum", bufs=2, space="PSUM") as mm_psum,
                tc.tile_pool(name="ev_cast", bufs=14) as ev_cast,
                tc.tile_pool(name="pyr", bufs=2) as pyr,
                tc.tile_pool(name="pyr5", bufs=3) as pyr5,
                tc.tile_pool(name="fin", bufs=3) as fin,
            ):

                def emit_mm_evac(t):
                    c0 = t * 128 if t < 8 else H + (t - 8) * 128
                    lhsT = normT[:, c0 : c0 + 128]
                    cas = []
                    for g in range(NG):
                        ps = mm_psum.tile([128, G], f32, tag="ps")
                        for s in range(G // 512):
                            cs = slice(g * G + s * 512, g * G + (s + 1) * 512)
                            nc.tensor.matmul(
                                ps[:, s * 512 : (s + 1) * 512],
                                lhsT, normT[:, cs],
                            )
                        ca = ev_cast.tile([128, G], bf16, tag="ca")
                        nc.scalar.activation(ca[:], ps[:], AF.Copy)
                        cas.append(ca)
                    return (t, cas)

                def emit_fold(state):
                    t, cas = state
                    l1b = pyr.tile([128, NG, G // 2], bf16, tag="l1b")
                    for g, ca in enumerate(cas):
                        nc.vector.tensor_max(
                            l1b[:, g, :],
                            ca[:, 0 : G // 2], ca[:, G // 2 : G],
                        )
                    f2 = pyr.tile([128, 4, G // 2], bf16, tag="f2")
                    for i in range(4):
                        nc.vector.tensor_max(
                            f2[:, i, :], l1b[:, 2 * i, :], l1b[:, 2 * i + 1, :]
                        )
                    f3 = pyr.tile([128, 2, G // 2], bf16, tag="f3")
                    for i in range(2):
                        nc.vector.tensor_max(
                            f3[:, i, :], f2[:, 2 * i, :], f2[:, 2 * i + 1, :]
                        )
                    f4 = pyr.tile([128, G // 2], bf16, tag="f4")
                    nc.vector.tensor_max(f4[:], f3[:, 0, :], f3[:, 1, :])
                    f5 = pyr5.tile([128, G // 4], bf16, tag="f5")
                    nc.vector.tensor_max(
                        f5[:], f4[:, 0 : G // 4], f4[:, G // 4 : G // 2]
                    )
                    return (t, f5)

                def emit_tail(state):
                    t, f5 = state
                    cand = fin.tile([128, 32], bf16, tag="cand")
                    for c in range(4):
                        nc.vector.max(
                            out=cand[:, c * 8 : (c + 1) * 8],
                            in_=f5[:, c * 128 : (c + 1) * 128],
                        )
                    top24 = fin.tile([128, 24], bf16, tag="top24")
                    cand2 = fin.tile([128, 32], bf16, tag="cand2")
                    cand3 = fin.tile([128, 32], bf16, tag="cand3")
                    nc.vector.max(out=top24[:, 0:8], in_=cand[:])
                    nc.vector.match_replace(
                        out=cand2[:], in_to_replace=top24[:, 0:8],
                        in_values=cand[:], imm_value=NEG,
                    )
                    nc.vector.max(out=top24[:, 8:16], in_=cand2[:])
                    nc.vector.match_replace(
                        out=cand3[:], in_to_replace=top24[:, 8:16],
                        in_values=cand2[:], imm_value=NEG,
                    )
                    nc.vector.max(out=top24[:, 16:24], in_=cand3[:])

                    osb = fin.tile([128, TOPK], f32, tag="osb")
                    nc.gpsimd.memset(osb[:, 0:1], 0.0)
                    nc.scalar.activation(
                        osb[:, 1:TOPK], top24[:, 1:TOPK], AF.Sigmoid
                    )
                    hh, band = (0, t) if t < 8 else (1, t - 8)
                    nc.sync.dma_start(
                        out_v[hh, band * 128 : (band + 1) * 128, :], osb[:]
                    )

                mm_prev = None
                fold_prev = None
                for t in range(T):
                    cur = emit_mm_evac(t)
                    if mm_prev is not None:
                        f = emit_fold(mm_prev)
                        if fold_prev is not None:
                            emit_tail(fold_prev)
                        fold_prev = f
                    mm_prev = cur
                f = emit_fold(mm_prev)
                emit_tail(fold_prev)
                emit_tail(f)

    nc.compile()
    return nc


def get_nc():
    if "nc" not in _CACHE:
        _CACHE["nc"] = _build_nc()
    return _CACHE["nc"]


def kernel(embeddings: np.ndarray) -> np.ndarray:
    emb = np.ascontiguousarray(np.asarray(embeddings, dtype=np.float32))
    assert emb.shape == (N, D), emb.shape
    nc = get_nc()
    in_maps = [
        {"embeddings": np.roll(emb, -i * R, axis=0)} for i in range(CORES)
    ]
    res = run_bass_kernel_spmd(nc, in_maps, core_ids=list(range(CORES)))
    _CACHE["last_results"] = res
    return np.concatenate(
        [res.results[i]["out"] for i in range(CORES)], axis=0
    ).astype(np.float32)
